# revision 2
# baseline (speedup 1.0000x reference)
"""DeepseekV3 MLA attention kernel for 8 Trainium2 NeuronCores — v3.

Sharding: 2-way data-parallel over batch x 4-way tensor-parallel over heads.
Core c handles batch b = c // 4 and heads [4*(c%4) .. 4*(c%4)+4).

Numerics split (validated vs reference in numpy):
  - fp8e4 DoubleRow for the softmax-damped paths: q projections, k
    projections, attention scores.  (weights pre-scaled x64 on host)
  - bf16 for the linear V-path: kv-latent production, v projection, e,
    softmax denominator, AV, output projection.  rel_err ~1.2e-2.

Perf structure:
  - matmuls sharing a stationary operand are emitted back-to-back
  - RoPE rotate-half via a second host-swapped weight matrix
  - no DRAM scratch; q/k/v/ao resident in SBUF
  - transposed output projection (w_o stationary x4 reuse); host transposes
  - engine balance: ACT = exp + psum evictions, DVE = rope/recip/ao +
    half of dmask + half of wo evictions, Pool = other half of dmask
"""

from contextlib import ExitStack
from dataclasses import dataclass

import numpy as np

import concourse.bacc as bacc
import concourse.mybir as mybir
import concourse.tile as tile

F32 = mybir.dt.float32
BF16 = mybir.dt.bfloat16
FP8 = mybir.dt.float8e4
DRM = mybir.MatmulPerfMode.DoubleRow
EXP = mybir.ActivationFunctionType.Exp
CPY = mybir.ActivationFunctionType.Copy

P = 128
WS = 64.0     # host scale for fp8 weights (keeps them out of subnormals)
IS = 1.0 / WS


@dataclass(frozen=True)
class Cfg:
    S: int = 2048
    HID: int = 2048
    QLR: int = 1536
    KVLR: int = 512
    NH_G: int = 4
    DN: int = 128
    DR: int = 64
    DV: int = 128

    @property
    def SCALE(self):
        return 1.0 / float(np.sqrt(self.DN + self.DR))


CFG = Cfg()


def build_nc(C: Cfg, reps: int = 1):
    nc = bacc.Bacc("TRN2", target_bir_lowering=False, debug=False, num_devices=8)
    HO = C.HID // P          # 16 hidden k-tiles
    KVC = C.KVLR // P        # 4 kv k-tiles
    NPAIR = C.NH_G // 2      # 2 rope pairs
    NVS = C.S // P           # 16 v s-tiles
    SC = C.S // 512          # 4 s-chunks

    # ---- kernel I/O ----
    hT8 = nc.dram_tensor("hT8", [C.HID, C.S], FP8, kind="ExternalInput").ap()
    hT16 = nc.dram_tensor("hT16", [C.HID, C.S], BF16, kind="ExternalInput").ap()
    # q weights (fp8 x64), cols: [qn 512 | qr-nat 256 | qr-swap 256]
    w_q8 = nc.dram_tensor("w_q8", [C.HID, 1024], FP8, kind="ExternalInput").ap()
    w_kva = nc.dram_tensor("w_kva", [C.HID, C.KVLR], BF16, kind="ExternalInput").ap()
    # k weights (fp8 x64), cols: [kbn 512 | kbr-nat 256 | kbr-swap 256]
    w_k8 = nc.dram_tensor("w_k8", [C.KVLR, 1024], FP8, kind="ExternalInput").ap()
    w_vb = nc.dram_tensor("w_vb", [C.KVLR, 512], BF16, kind="ExternalInput").ap()
    w_ob = nc.dram_tensor("w_ob", [C.NH_G * C.DV, C.HID], BF16,
                          kind="ExternalInput").ap()
    cos2 = nc.dram_tensor("cos2", [P, C.S], F32, kind="ExternalInput").ap()
    ssin2 = nc.dram_tensor("ssin2", [P, C.S], F32, kind="ExternalInput").ap()
    dmask = nc.dram_tensor("dmask", [512, 512], BF16, kind="ExternalInput").ap()
    outpT = nc.dram_tensor("outpT", [C.HID, C.S], BF16, kind="ExternalOutput").ap()

    with tile.TileContext(nc) as tc:
        for rep in range(reps):
            with ExitStack() as tctx:
                per_pool = tctx.enter_context(tc.tile_pool(name=f"per{rep}", bufs=1))
                # persistent activations
                q2_sb = per_pool.tile([P, C.NH_G, 2, C.S], FP8)
                kT2_sb = per_pool.tile([P, C.NH_G, 2, C.S], FP8)
                kvc8_sb = per_pool.tile([P, KVC, C.S], FP8)
                kvc16_sb = per_pool.tile([P, KVC, C.S], BF16)
                v_sb = per_pool.tile([P, NVS, C.NH_G * C.DV], BF16)
                ao16_sb = per_pool.tile([P, C.NH_G, C.S], BF16)
                dm_sb = per_pool.tile([P, 4, 512], BF16)
                wo_sb = per_pool.tile([P, C.NH_G, C.HID], BF16)
                ones_sb = per_pool.tile([P, P], BF16)
                nc.sync.dma_start(
                    out=dm_sb[:], in_=dmask.rearrange("(j ki) q -> ki j q", ki=P))
                nc.sync.dma_start(
                    out=wo_sb[:], in_=w_ob.rearrange("(j d) o -> d j o", d=P))
                nc.vector.memset(ones_sb[:], 1.0)
                # zero the unused rope halves of kT2 once
                for h in range(C.NH_G):
                    r0 = 64 if h % 2 == 0 else 0
                    nc.vector.memset(kT2_sb[r0:r0 + 64, h, 1, :], 0.0)

                tab_pool = tctx.enter_context(tc.tile_pool(name=f"tab{rep}", bufs=1))
                cos_sb = tab_pool.tile([P, C.S], F32)
                ssin_sb = tab_pool.tile([P, C.S], F32)
                nc.sync.dma_start(out=cos_sb[:], in_=cos2)
                nc.sync.dma_start(out=ssin_sb[:], in_=ssin2)

                # ===== Phase 1a: kv latent (bf16, accurate V-path) =====
                with ExitStack() as ctx:
                    wv_pool = ctx.enter_context(tc.tile_pool(name=f"wv{rep}", bufs=1))
                    h16_pool = ctx.enter_context(tc.tile_pool(name=f"h16{rep}", bufs=1))
                    psa = ctx.enter_context(
                        tc.tile_pool(name=f"psa{rep}", bufs=8, space="PSUM"))
                    wkva_sb = wv_pool.tile([P, HO, C.KVLR], BF16)
                    nc.sync.dma_start(
                        out=wkva_sb[:], in_=w_kva.rearrange("(ho hi) c -> hi ho c", hi=P))
                    h16_r = hT16.rearrange("(ho hi) s -> hi ho s", hi=P)
                    for half in range(2):
                        s0h = half * 1024
                        h16_sb = h16_pool.tile([P, HO, 1024], BF16, tag="h16")
                        nc.sync.dma_start(out=h16_sb[:],
                                          in_=h16_r[:, :, s0h:s0h + 1024])
                        for m in range(KVC):
                            pss = [psa.tile([P, 512], F32, tag="pa", name=f"pa{i}")
                                   for i in range(2)]
                            for kt in range(HO):
                                w_ap = wkva_sb[:, kt, m * P:(m + 1) * P]
                                for i, ps in enumerate(pss):
                                    nc.tensor.matmul(
                                        ps[:], w_ap,
                                        h16_sb[:, kt, i * 512:(i + 1) * 512],
                                        start=(kt == 0), stop=(kt == HO - 1))
                            for i, ps in enumerate(pss):
                                s0 = s0h + i * 512
                                nc.scalar.activation(
                                    kvc16_sb[:, m, s0:s0 + 512], ps[:], CPY)
                                nc.scalar.activation(
                                    kvc8_sb[:, m, s0:s0 + 512], ps[:], CPY)

                # ===== Phase 1b: fused q projections (fp8 DR) =====
                with ExitStack() as ctx:
                    w1_pool = ctx.enter_context(tc.tile_pool(name=f"w1{rep}", bufs=1))
                    ht_pool = ctx.enter_context(tc.tile_pool(name=f"ht{rep}", bufs=1))
                    rp_pool = ctx.enter_context(tc.tile_pool(name=f"rp{rep}", bufs=4))
                    ps1 = ctx.enter_context(
                        tc.tile_pool(name=f"ps1{rep}", bufs=8, space="PSUM"))

                    w1_sb = w1_pool.tile([P, HO, 1024], FP8)
                    nc.sync.dma_start(
                        out=w1_sb[:], in_=w_q8.rearrange("(ho hi) c -> hi ho c", hi=P))
                    ht_sb = ht_pool.tile([P, HO, C.S], FP8)
                    ht_r = hT8.rearrange("(ho hi) s -> hi ho s", hi=P)
                    for sc in range(SC):
                        nc.sync.dma_start(
                            out=ht_sb[:, :, sc * 512:(sc + 1) * 512],
                            in_=ht_r[:, :, sc * 512:(sc + 1) * 512])

                    def p1_accum(col0, nsc, sc0=0):
                        pss = [ps1.tile([P, 512], F32, tag="p1", name=f"p1_{i}")
                               for i in range(nsc)]
                        for kp in range(HO // 2):
                            w_ap = w1_sb[:, 2 * kp:2 * kp + 2, col0:col0 + P]
                            for i, ps in enumerate(pss):
                                s0 = (sc0 + i) * 512
                                nc.tensor.matmul(
                                    ps[:], w_ap,
                                    ht_sb[:, 2 * kp:2 * kp + 2, s0:s0 + 512],
                                    start=(kp == 0), stop=(kp == HO // 2 - 1),
                                    perf_mode=DRM)
                        return pss

                    for h in range(C.NH_G):
                        pss = p1_accum(h * P, SC)
                        for i, ps in enumerate(pss):
                            nc.scalar.activation(
                                q2_sb[:, h, 0, i * 512:(i + 1) * 512], ps[:],
                                CPY, scale=IS)
                    for pr in range(NPAIR):
                        for half in range(2):
                            nat = p1_accum(512 + pr * P, 2, sc0=half * 2)
                            swp = p1_accum(768 + pr * P, 2, sc0=half * 2)
                            for i in range(2):
                                s0 = (half * 2 + i) * 512
                                m1 = rp_pool.tile([P, 512], F32, tag="rp1")
                                nc.vector.tensor_mul(
                                    m1[:], nat[i][:], cos_sb[:, s0:s0 + 512])
                                m2 = rp_pool.tile([P, 512], F32, tag="rp2")
                                nc.vector.tensor_mul(
                                    m2[:], swp[i][:], ssin_sb[:, s0:s0 + 512])
                                for hh in range(2):
                                    nc.vector.tensor_add(
                                        q2_sb[:, 2 * pr + hh, 1, s0:s0 + 512],
                                        m1[:], m2[:])

                # ===== Phase 2: k projections (fp8 DR) + v (bf16) =====
                with ExitStack() as ctx:
                    w2_pool = ctx.enter_context(tc.tile_pool(name=f"w2{rep}", bufs=1))
                    rp_pool = ctx.enter_context(tc.tile_pool(name=f"rq{rep}", bufs=4))
                    ps2 = ctx.enter_context(
                        tc.tile_pool(name=f"ps2{rep}", bufs=6, space="PSUM"))
                    ps2v = ctx.enter_context(
                        tc.tile_pool(name=f"ps2v{rep}", bufs=2, space="PSUM"))

                    w2_sb = w2_pool.tile([P, KVC, 1024], FP8)
                    nc.sync.dma_start(
                        out=w2_sb[:], in_=w_k8.rearrange("(co ci) m -> ci co m", ci=P))
                    wvb_sb = w2_pool.tile([P, KVC, 512], BF16)
                    nc.sync.dma_start(
                        out=wvb_sb[:], in_=w_vb.rearrange("(co ci) m -> ci co m", ci=P))

                    def p2_accum(col0, nsc, sc0=0):
                        pss = [ps2.tile([P, 512], F32, tag="p2", name=f"p2_{i}")
                               for i in range(nsc)]
                        for kp in range(KVC // 2):
                            w_ap = w2_sb[:, 2 * kp:2 * kp + 2, col0:col0 + P]
                            for i, ps in enumerate(pss):
                                s0 = (sc0 + i) * 512
                                nc.tensor.matmul(
                                    ps[:], w_ap,
                                    kvc8_sb[:, 2 * kp:2 * kp + 2, s0:s0 + 512],
                                    start=(kp == 0), stop=(kp == KVC // 2 - 1),
                                    perf_mode=DRM)
                        return pss

                    for h in range(C.NH_G):
                        pss = p2_accum(h * P, SC)
                        for i, ps in enumerate(pss):
                            nc.scalar.activation(
                                kT2_sb[:, h, 0, i * 512:(i + 1) * 512], ps[:],
                                CPY, scale=IS)
                    for pr in range(NPAIR):
                        for half in range(2):
                            nat = p2_accum(512 + pr * P, 2, sc0=half * 2)
                            swp = p2_accum(768 + pr * P, 2, sc0=half * 2)
                            for i in range(2):
                                s0 = (half * 2 + i) * 512
                                m1 = rp_pool.tile([P, 512], F32, tag="rp1")
                                nc.vector.tensor_mul(
                                    m1[:], nat[i][:], cos_sb[:, s0:s0 + 512])
                                m2 = rp_pool.tile([P, 512], F32, tag="rp2")
                                nc.vector.tensor_mul(
                                    m2[:], swp[i][:], ssin_sb[:, s0:s0 + 512])
                                nc.vector.tensor_add(
                                    kT2_sb[0:64, 2 * pr, 1, s0:s0 + 512],
                                    m1[0:64, :], m2[0:64, :])
                                nc.vector.tensor_add(
                                    kT2_sb[64:128, 2 * pr + 1, 1, s0:s0 + 512],
                                    m1[64:128, :], m2[64:128, :])
                    # v from kvc16 (bf16)
                    for st in range(NVS):
                        psv = ps2v.tile([P, 512], F32, tag="p2v")
                        for kt in range(KVC):
                            nc.tensor.matmul(
                                psv[:], kvc16_sb[:, kt, st * P:(st + 1) * P],
                                wvb_sb[:, kt, :],
                                start=(kt == 0), stop=(kt == KVC - 1))
                        nc.scalar.activation(v_sb[:, st, :], psv[:], CPY)

                # ===== Phase 3: attention =====
                with ExitStack() as ctx:
                    e_pool = ctx.enter_context(tc.tile_pool(name=f"ae{rep}", bufs=6))
                    d_pool = ctx.enter_context(tc.tile_pool(name=f"ad{rep}", bufs=2))
                    ps_s = ctx.enter_context(
                        tc.tile_pool(name=f"apss{rep}", bufs=3, space="PSUM"))
                    ps_d = ctx.enter_context(
                        tc.tile_pool(name=f"apsd{rep}", bufs=2, space="PSUM"))
                    ps_o = ctx.enter_context(
                        tc.tile_pool(name=f"apso{rep}", bufs=2, space="PSUM"))

                    nmask = 0
                    for h in range(C.NH_G):
                        for H in range(2):
                            qts = [2 * H, 2 * H + 1]
                            kb_end = (2 * H + 2) * 4
                            psd = {qt: ps_d.tile([P, 512], F32, tag="psd",
                                                 name=f"psd{qt}") for qt in qts}
                            pso = {qt: ps_o.tile([P, 512], F32, tag="pso",
                                                 name=f"pso{qt}") for qt in qts}
                            pend = []
                            for kb in range(kb_end):
                                qlist = [qt for qt in qts if (qt + 1) * 4 > kb]
                                cur = []
                                for qt in qlist:
                                    pss = ps_s.tile([P, 512], F32, tag="pss",
                                                    name="pss")
                                    q0 = qt * 512
                                    nc.tensor.matmul(
                                        pss[:],
                                        kT2_sb[:, h, :, kb * P:(kb + 1) * P],
                                        q2_sb[:, h, :, q0:q0 + 512],
                                        start=True, stop=True, perf_mode=DRM)
                                    e16 = e_pool.tile([P, 512], BF16, tag="e",
                                                      name="e16")
                                    nc.scalar.activation(e16[:], pss[:], EXP,
                                                         scale=C.SCALE)
                                    if kb // 4 == qt:
                                        eng = nc.vector if nmask % 2 else nc.gpsimd
                                        nmask += 1
                                        eng.tensor_mul(
                                            e16[:], e16[:], dm_sb[:, kb % 4, :])
                                    cur.append((qt, kb, e16))
                                # denominators grouped (shared ones stationary),
                                # then AV grouped (shared v stationary)
                                for qt, kbp, e16 in pend:
                                    nc.tensor.matmul(
                                        psd[qt][:], ones_sb[:], e16[:],
                                        start=(kbp == 0),
                                        stop=(kbp == (qt + 1) * 4 - 1))
                                for qt, kbp, e16 in pend:
                                    nc.tensor.matmul(
                                        pso[qt][:],
                                        v_sb[:, kbp, h * C.DV:(h + 1) * C.DV],
                                        e16[:], start=(kbp == 0),
                                        stop=(kbp == (qt + 1) * 4 - 1))
                                pend = cur
                            for qt, kbp, e16 in pend:
                                nc.tensor.matmul(
                                    psd[qt][:], ones_sb[:], e16[:],
                                    start=(kbp == 0), stop=True)
                            for qt, kbp, e16 in pend:
                                nc.tensor.matmul(
                                    pso[qt][:],
                                    v_sb[:, kbp, h * C.DV:(h + 1) * C.DV],
                                    e16[:], start=(kbp == 0), stop=True)

                            for qt in qts:
                                q0 = qt * 512
                                rec = d_pool.tile([P, 512], F32, tag="rec")
                                nc.vector.reciprocal(rec[:], psd[qt][:])
                                nc.vector.tensor_mul(
                                    ao16_sb[:, h, q0:q0 + 512], pso[qt][:], rec[:])

                # ===== output projection (transposed: w_o stationary) =====
                with ExitStack() as ctx:
                    oev_pool = ctx.enter_context(tc.tile_pool(name=f"oe{rep}", bufs=4))
                    ps_w = ctx.enter_context(
                        tc.tile_pool(name=f"apsw{rep}", bufs=8, space="PSUM"))
                    for hc in range(C.HID // P):
                        psw = [ps_w.tile([P, 512], F32, tag="psw", name=f"psw{i}")
                               for i in range(SC)]
                        for j in range(C.NH_G):
                            w_ap = wo_sb[:, j, hc * P:(hc + 1) * P]
                            for qc in range(SC):
                                nc.tensor.matmul(
                                    psw[qc][:], w_ap,
                                    ao16_sb[:, j, qc * 512:(qc + 1) * 512],
                                    start=(j == 0), stop=(j == C.NH_G - 1))
                        for qc in range(SC):
                            oev = oev_pool.tile([P, 512], BF16, tag="oev")
                            if qc % 2 == 0:
                                nc.vector.tensor_copy(oev[:], psw[qc][:])
                            else:
                                nc.scalar.copy(oev[:], psw[qc][:])
                            nc.sync.dma_start(
                                out=outpT[hc * P:(hc + 1) * P,
                                          qc * 512:(qc + 1) * 512],
                                in_=oev[:])

    nc.compile()
    return nc


def rope_tables(C: Cfg):
    """cos2/ssin2 [128, S] f32, two stacked 64-row blocks, pre-scaled by 1/WS."""
    inv = 1.0 / (10000.0 ** (np.arange(0, C.DR, 2, dtype=np.float64) / C.DR))
    freqs = np.arange(C.S, dtype=np.float64)[:, None] * inv[None, :]  # [S, 32]
    emb = np.concatenate([freqs, freqs], axis=1)  # [S, 64]
    cos = np.cos(emb).T
    sin = np.sin(emb).T
    ssin = sin.copy()
    ssin[: C.DR // 2] = -ssin[: C.DR // 2]
    cos2 = np.concatenate([cos, cos], axis=0) * IS
    ssin2 = np.concatenate([ssin, ssin], axis=0) * IS
    return (np.ascontiguousarray(cos2.astype(np.float32)),
            np.ascontiguousarray(ssin2.astype(np.float32)))


def swap32(w):
    """Swap the two 32-col halves of every 64-col group (rotate_half source)."""
    n = w.shape[1]
    return w.reshape(w.shape[0], n // 64, 2, 32)[:, :, ::-1, :].reshape(w.shape[0], n)


FP8NP = mybir.dt.np(FP8)
BF16NP = mybir.dt.np(BF16)


def host_inputs(C: Cfg, inputs: dict, core: int):
    NH = inputs["w_q_nope"].shape[1] // C.DN
    groups = NH // C.NH_G
    b = core // groups
    g = core % groups
    hs = slice(g * C.NH_G, (g + 1) * C.NH_G)

    f64 = lambda x: np.asarray(x, dtype=np.float64)
    fp8 = lambda x: np.ascontiguousarray(np.asarray(x, dtype=np.float32).astype(FP8NP))
    bf16 = lambda x: np.ascontiguousarray(
        np.asarray(x, dtype=np.float32).astype(BF16NP))

    hT = inputs["hidden_states"][b].T
    w_q_a = f64(inputs["w_q_a"])
    w_qbn = f64(inputs["w_q_nope"].reshape(C.QLR, NH, C.DN)[:, hs].reshape(C.QLR, -1))
    w_qbr = f64(inputs["w_q_rope"].reshape(C.QLR, NH, C.DR)[:, hs].reshape(C.QLR, -1))
    qn = w_q_a @ w_qbn          # [HID, 512]
    qr = w_q_a @ w_qbr          # [HID, 256]
    w_q8 = fp8(np.concatenate([qn, qr, swap32(qr)], axis=1) * WS)

    w_kbn = f64(inputs["w_k_nope"].reshape(C.KVLR, NH, C.DN)[:, hs].reshape(C.KVLR, -1))
    w_kbr = f64(inputs["w_k_rope"].reshape(C.KVLR, NH, C.DR)[:, hs].reshape(C.KVLR, -1))
    w_k8 = fp8(np.concatenate([w_kbn, w_kbr, swap32(w_kbr)], axis=1) * WS)
    w_vb = bf16(inputs["w_v"].reshape(C.KVLR, NH, C.DV)[:, hs].reshape(C.KVLR, -1))

    w_ob = bf16(inputs["w_o"].reshape(NH, C.DV, C.HID)[hs].reshape(-1, C.HID))
    cos2, ssin2 = rope_tables(C)
    cm = np.asarray(inputs["causal_mask"])[0, 0]
    dmask = bf16(cm[-512:, -512:].T)
    return {
        "hT8": fp8(hT), "hT16": bf16(hT), "w_q8": w_q8,
        "w_kva": bf16(inputs["w_kv_a"]), "w_k8": w_k8, "w_vb": w_vb,
        "w_ob": w_ob, "cos2": cos2, "ssin2": ssin2, "dmask": dmask,
    }


_NC_CACHE = {}


def kernel(**inputs) -> np.ndarray:
    from concourse.bass_utils import run_bass_kernel_spmd

    C = CFG
    if "nc" not in _NC_CACHE:
        _NC_CACHE["nc"] = build_nc(C)
    nc = _NC_CACHE["nc"]

    in_maps = [host_inputs(C, inputs, c) for c in range(8)]
    res = run_bass_kernel_spmd(nc, in_maps, core_ids=list(range(8)))

    B = inputs["hidden_states"].shape[0]
    groups = 8 // B
    out = np.zeros((B, C.S, C.HID), dtype=np.float32)
    for c in range(8):
        out[c // groups] += np.asarray(res.results[c]["outpT"],
                                       dtype=np.float32).T
    return out


# revision 3
# speedup vs baseline: 1.0558x; 1.0558x over previous
"""DeepseekV3 MLA attention kernel for 8 Trainium2 NeuronCores — v3.

Sharding: 2-way data-parallel over batch x 4-way tensor-parallel over heads.
Core c handles batch b = c // 4 and heads [4*(c%4) .. 4*(c%4)+4).

Numerics split (validated vs reference in numpy):
  - fp8e4 DoubleRow for the softmax-damped paths: q projections, k
    projections, attention scores.  (weights pre-scaled x64 on host)
  - bf16 for the linear V-path: kv-latent production, v projection, e,
    softmax denominator, AV, output projection.  rel_err ~1.2e-2.

Perf structure:
  - matmuls sharing a stationary operand are emitted back-to-back
  - RoPE rotate-half via a second host-swapped weight matrix
  - no DRAM scratch; q/k/v/ao resident in SBUF
  - transposed output projection (w_o stationary x4 reuse); host transposes
  - engine balance: ACT = exp + psum evictions, DVE = rope/recip/ao +
    half of dmask + half of wo evictions, Pool = other half of dmask
"""

from contextlib import ExitStack
from dataclasses import dataclass

import numpy as np

import concourse.bacc as bacc
import concourse.mybir as mybir
import concourse.tile as tile

F32 = mybir.dt.float32
BF16 = mybir.dt.bfloat16
FP8 = mybir.dt.float8e4
DRM = mybir.MatmulPerfMode.DoubleRow
EXP = mybir.ActivationFunctionType.Exp
CPY = mybir.ActivationFunctionType.Copy

P = 128
WS = 64.0     # host scale for fp8 weights (keeps them out of subnormals)
IS = 1.0 / WS


@dataclass(frozen=True)
class Cfg:
    S: int = 2048
    HID: int = 2048
    QLR: int = 1536
    KVLR: int = 512
    NH_G: int = 4
    DN: int = 128
    DR: int = 64
    DV: int = 128

    @property
    def SCALE(self):
        return 1.0 / float(np.sqrt(self.DN + self.DR))


CFG = Cfg()


def build_nc(C: Cfg, reps: int = 1):
    nc = bacc.Bacc("TRN2", target_bir_lowering=False, debug=False, num_devices=8)
    HO = C.HID // P          # 16 hidden k-tiles
    KVC = C.KVLR // P        # 4 kv k-tiles
    NPAIR = C.NH_G // 2      # 2 rope pairs
    NVS = C.S // P           # 16 v s-tiles
    SC = C.S // 512          # 4 s-chunks

    # ---- kernel I/O ----
    hT8 = nc.dram_tensor("hT8", [C.HID, C.S], FP8, kind="ExternalInput").ap()
    hT16 = nc.dram_tensor("hT16", [C.HID, C.S], BF16, kind="ExternalInput").ap()
    # q weights (fp8 x64), cols: [qn 512 | qr 256]
    w_q8 = nc.dram_tensor("w_q8", [C.HID, 768], FP8, kind="ExternalInput").ap()
    w_kva = nc.dram_tensor("w_kva", [C.HID, C.KVLR], BF16, kind="ExternalInput").ap()
    # k weights (fp8 x64), cols: [kbn 512 | kbr 256]
    w_k8 = nc.dram_tensor("w_k8", [C.KVLR, 768], FP8, kind="ExternalInput").ap()
    w_vb = nc.dram_tensor("w_vb", [C.KVLR, 512], BF16, kind="ExternalInput").ap()
    w_ob = nc.dram_tensor("w_ob", [C.NH_G * C.DV, C.HID], BF16,
                          kind="ExternalInput").ap()
    cos2 = nc.dram_tensor("cos2", [P, C.S], F32, kind="ExternalInput").ap()
    ssin2 = nc.dram_tensor("ssin2", [P, C.S], F32, kind="ExternalInput").ap()
    dmask = nc.dram_tensor("dmask", [512, 512], BF16, kind="ExternalInput").ap()
    outpT = nc.dram_tensor("outpT", [C.HID, C.S], BF16, kind="ExternalOutput").ap()

    with tile.TileContext(nc) as tc:
        for rep in range(reps):
            with ExitStack() as tctx:
                per_pool = tctx.enter_context(tc.tile_pool(name=f"per{rep}", bufs=1))
                # persistent activations
                q2_sb = per_pool.tile([P, C.NH_G, 2, C.S], FP8)
                kT2_sb = per_pool.tile([P, C.NH_G, 2, C.S], FP8)
                kvc8_sb = per_pool.tile([P, KVC, C.S], FP8)
                kvc16_sb = per_pool.tile([P, KVC, C.S], BF16)
                v_sb = per_pool.tile([P, NVS, C.NH_G * C.DV], BF16)
                ao16_sb = per_pool.tile([P, C.NH_G, C.S], BF16)
                dm_sb = per_pool.tile([P, 4, 512], BF16)
                wo_sb = per_pool.tile([P, C.NH_G, C.HID], BF16)
                ones_sb = per_pool.tile([P, P], BF16)
                nc.sync.dma_start(
                    out=dm_sb[:], in_=dmask.rearrange("(j ki) q -> ki j q", ki=P))
                nc.sync.dma_start(
                    out=wo_sb[:], in_=w_ob.rearrange("(j d) o -> d j o", d=P))
                nc.vector.memset(ones_sb[:], 1.0)
                # zero the unused rope halves of kT2 once
                for h in range(C.NH_G):
                    r0 = 64 if h % 2 == 0 else 0
                    nc.vector.memset(kT2_sb[r0:r0 + 64, h, 1, :], 0.0)

                tab_pool = tctx.enter_context(tc.tile_pool(name=f"tab{rep}", bufs=1))
                cos_sb = tab_pool.tile([P, C.S], F32)
                ssin_sb = tab_pool.tile([P, C.S], F32)
                nc.sync.dma_start(out=cos_sb[:], in_=cos2)
                nc.sync.dma_start(out=ssin_sb[:], in_=ssin2)


                def rope_mats(rp_pool, ps_nat, s0):
                    """psum rope rows -> (m1, qs) f32 tiles: m1=x*cos, qs=rot."""
                    tmp = rp_pool.tile([P, 512], F32, tag="rt", name="rt")
                    nc.vector.tensor_copy(tmp[:], ps_nat[:])
                    qs = rp_pool.tile([P, 512], F32, tag="rs", name="rs")
                    for g in range(4):
                        nc.sync.dma_start(
                            out=qs[(g ^ 1) * 32:(g ^ 1) * 32 + 32, :],
                            in_=tmp[g * 32:(g + 1) * 32, :])
                    m1 = rp_pool.tile([P, 512], F32, tag="rp1", name="m1")
                    nc.vector.tensor_mul(m1[:], ps_nat[:], cos_sb[:, s0:s0 + 512])
                    nc.vector.tensor_mul(qs[:], qs[:], ssin_sb[:, s0:s0 + 512])
                    return m1, qs
                # ===== Phase 1a: kv latent (bf16, accurate V-path) =====
                with ExitStack() as ctx:
                    wv_pool = ctx.enter_context(tc.tile_pool(name=f"wv{rep}", bufs=1))
                    h16_pool = ctx.enter_context(tc.tile_pool(name=f"h16{rep}", bufs=1))
                    psa = ctx.enter_context(
                        tc.tile_pool(name=f"psa{rep}", bufs=8, space="PSUM"))
                    wkva_sb = wv_pool.tile([P, HO, C.KVLR], BF16)
                    nc.sync.dma_start(
                        out=wkva_sb[:], in_=w_kva.rearrange("(ho hi) c -> hi ho c", hi=P))
                    h16_r = hT16.rearrange("(ho hi) s -> hi ho s", hi=P)
                    for half in range(2):
                        s0h = half * 1024
                        h16_sb = h16_pool.tile([P, HO, 1024], BF16, tag="h16")
                        nc.sync.dma_start(out=h16_sb[:],
                                          in_=h16_r[:, :, s0h:s0h + 1024])
                        for m in range(KVC):
                            pss = [psa.tile([P, 512], F32, tag="pa", name=f"pa{i}")
                                   for i in range(2)]
                            for kt in range(HO):
                                w_ap = wkva_sb[:, kt, m * P:(m + 1) * P]
                                for i, ps in enumerate(pss):
                                    nc.tensor.matmul(
                                        ps[:], w_ap,
                                        h16_sb[:, kt, i * 512:(i + 1) * 512],
                                        start=(kt == 0), stop=(kt == HO - 1))
                            for i, ps in enumerate(pss):
                                s0 = s0h + i * 512
                                nc.scalar.activation(
                                    kvc16_sb[:, m, s0:s0 + 512], ps[:], CPY)
                                nc.scalar.activation(
                                    kvc8_sb[:, m, s0:s0 + 512], ps[:], CPY)

                # ===== Phase 1b: fused q projections (fp8 DR) =====
                with ExitStack() as ctx:
                    w1_pool = ctx.enter_context(tc.tile_pool(name=f"w1{rep}", bufs=1))
                    ht_pool = ctx.enter_context(tc.tile_pool(name=f"ht{rep}", bufs=1))
                    rp_pool = ctx.enter_context(tc.tile_pool(name=f"rp{rep}", bufs=4))
                    ps1 = ctx.enter_context(
                        tc.tile_pool(name=f"ps1{rep}", bufs=8, space="PSUM"))

                    w1_sb = w1_pool.tile([P, HO, 768], FP8)
                    nc.sync.dma_start(
                        out=w1_sb[:], in_=w_q8.rearrange("(ho hi) c -> hi ho c", hi=P))
                    ht_sb = ht_pool.tile([P, HO, C.S], FP8)
                    ht_r = hT8.rearrange("(ho hi) s -> hi ho s", hi=P)
                    for sc in range(SC):
                        nc.sync.dma_start(
                            out=ht_sb[:, :, sc * 512:(sc + 1) * 512],
                            in_=ht_r[:, :, sc * 512:(sc + 1) * 512])

                    def p1_accum(col0, nsc, sc0=0):
                        pss = [ps1.tile([P, 512], F32, tag="p1", name=f"p1_{i}")
                               for i in range(nsc)]
                        for kp in range(HO // 2):
                            w_ap = w1_sb[:, 2 * kp:2 * kp + 2, col0:col0 + P]
                            for i, ps in enumerate(pss):
                                s0 = (sc0 + i) * 512
                                nc.tensor.matmul(
                                    ps[:], w_ap,
                                    ht_sb[:, 2 * kp:2 * kp + 2, s0:s0 + 512],
                                    start=(kp == 0), stop=(kp == HO // 2 - 1),
                                    perf_mode=DRM)
                        return pss

                    for h in range(C.NH_G):
                        pss = p1_accum(h * P, SC)
                        for i, ps in enumerate(pss):
                            nc.scalar.activation(
                                q2_sb[:, h, 0, i * 512:(i + 1) * 512], ps[:],
                                CPY, scale=IS)
                    for pr in range(NPAIR):
                        nat = p1_accum(512 + pr * P, SC)
                        for i in range(SC):
                            s0 = i * 512
                            m1, qs = rope_mats(rp_pool, nat[i], s0)
                            for hh in range(2):
                                nc.vector.tensor_add(
                                    q2_sb[:, 2 * pr + hh, 1, s0:s0 + 512],
                                    m1[:], qs[:])

                # ===== Phase 2: k projections (fp8 DR) + v (bf16) =====
                with ExitStack() as ctx:
                    w2_pool = ctx.enter_context(tc.tile_pool(name=f"w2{rep}", bufs=1))
                    rp_pool = ctx.enter_context(tc.tile_pool(name=f"rq{rep}", bufs=4))
                    ps2 = ctx.enter_context(
                        tc.tile_pool(name=f"ps2{rep}", bufs=6, space="PSUM"))
                    ps2v = ctx.enter_context(
                        tc.tile_pool(name=f"ps2v{rep}", bufs=2, space="PSUM"))

                    w2_sb = w2_pool.tile([P, KVC, 768], FP8)
                    nc.sync.dma_start(
                        out=w2_sb[:], in_=w_k8.rearrange("(co ci) m -> ci co m", ci=P))
                    wvb_sb = w2_pool.tile([P, KVC, 512], BF16)
                    nc.sync.dma_start(
                        out=wvb_sb[:], in_=w_vb.rearrange("(co ci) m -> ci co m", ci=P))

                    def p2_accum(col0, nsc, sc0=0):
                        pss = [ps2.tile([P, 512], F32, tag="p2", name=f"p2_{i}")
                               for i in range(nsc)]
                        for kp in range(KVC // 2):
                            w_ap = w2_sb[:, 2 * kp:2 * kp + 2, col0:col0 + P]
                            for i, ps in enumerate(pss):
                                s0 = (sc0 + i) * 512
                                nc.tensor.matmul(
                                    ps[:], w_ap,
                                    kvc8_sb[:, 2 * kp:2 * kp + 2, s0:s0 + 512],
                                    start=(kp == 0), stop=(kp == KVC // 2 - 1),
                                    perf_mode=DRM)
                        return pss

                    for h in range(C.NH_G):
                        pss = p2_accum(h * P, SC)
                        for i, ps in enumerate(pss):
                            nc.scalar.activation(
                                kT2_sb[:, h, 0, i * 512:(i + 1) * 512], ps[:],
                                CPY, scale=IS)
                    for pr in range(NPAIR):
                        nat = p2_accum(512 + pr * P, SC)
                        for i in range(SC):
                            s0 = i * 512
                            m1, qs = rope_mats(rp_pool, nat[i], s0)
                            nc.vector.tensor_add(
                                kT2_sb[0:64, 2 * pr, 1, s0:s0 + 512],
                                m1[0:64, :], qs[0:64, :])
                            nc.vector.tensor_add(
                                kT2_sb[64:128, 2 * pr + 1, 1, s0:s0 + 512],
                                m1[64:128, :], qs[64:128, :])
                    # v from kvc16 (bf16)
                    for st in range(NVS):
                        psv = ps2v.tile([P, 512], F32, tag="p2v")
                        for kt in range(KVC):
                            nc.tensor.matmul(
                                psv[:], kvc16_sb[:, kt, st * P:(st + 1) * P],
                                wvb_sb[:, kt, :],
                                start=(kt == 0), stop=(kt == KVC - 1))
                        nc.scalar.activation(v_sb[:, st, :], psv[:], CPY)

                # ===== Phase 3: attention =====
                with ExitStack() as ctx:
                    e_pool = ctx.enter_context(tc.tile_pool(name=f"ae{rep}", bufs=6))
                    d_pool = ctx.enter_context(tc.tile_pool(name=f"ad{rep}", bufs=2))
                    ps_s = ctx.enter_context(
                        tc.tile_pool(name=f"apss{rep}", bufs=3, space="PSUM"))
                    ps_d = ctx.enter_context(
                        tc.tile_pool(name=f"apsd{rep}", bufs=2, space="PSUM"))
                    ps_o = ctx.enter_context(
                        tc.tile_pool(name=f"apso{rep}", bufs=2, space="PSUM"))

                    nmask = 0
                    for h in range(C.NH_G):
                        for H in range(2):
                            qts = [2 * H, 2 * H + 1]
                            kb_end = (2 * H + 2) * 4
                            psd = {qt: ps_d.tile([P, 512], F32, tag="psd",
                                                 name=f"psd{qt}") for qt in qts}
                            pso = {qt: ps_o.tile([P, 512], F32, tag="pso",
                                                 name=f"pso{qt}") for qt in qts}
                            pend = []
                            for kb in range(kb_end):
                                qlist = [qt for qt in qts if (qt + 1) * 4 > kb]
                                cur = []
                                for qt in qlist:
                                    pss = ps_s.tile([P, 512], F32, tag="pss",
                                                    name="pss")
                                    q0 = qt * 512
                                    nc.tensor.matmul(
                                        pss[:],
                                        kT2_sb[:, h, :, kb * P:(kb + 1) * P],
                                        q2_sb[:, h, :, q0:q0 + 512],
                                        start=True, stop=True, perf_mode=DRM)
                                    e16 = e_pool.tile([P, 512], BF16, tag="e",
                                                      name="e16")
                                    nc.scalar.activation(e16[:], pss[:], EXP,
                                                         scale=C.SCALE)
                                    if kb // 4 == qt:
                                        eng = nc.vector if nmask % 2 else nc.gpsimd
                                        nmask += 1
                                        eng.tensor_mul(
                                            e16[:], e16[:], dm_sb[:, kb % 4, :])
                                    cur.append((qt, kb, e16))
                                # denominators grouped (shared ones stationary),
                                # then AV grouped (shared v stationary)
                                for qt, kbp, e16 in pend:
                                    nc.tensor.matmul(
                                        psd[qt][:], ones_sb[:], e16[:],
                                        start=(kbp == 0),
                                        stop=(kbp == (qt + 1) * 4 - 1))
                                for qt, kbp, e16 in pend:
                                    nc.tensor.matmul(
                                        pso[qt][:],
                                        v_sb[:, kbp, h * C.DV:(h + 1) * C.DV],
                                        e16[:], start=(kbp == 0),
                                        stop=(kbp == (qt + 1) * 4 - 1))
                                pend = cur
                            for qt, kbp, e16 in pend:
                                nc.tensor.matmul(
                                    psd[qt][:], ones_sb[:], e16[:],
                                    start=(kbp == 0), stop=True)
                            for qt, kbp, e16 in pend:
                                nc.tensor.matmul(
                                    pso[qt][:],
                                    v_sb[:, kbp, h * C.DV:(h + 1) * C.DV],
                                    e16[:], start=(kbp == 0), stop=True)

                            for qt in qts:
                                q0 = qt * 512
                                rec = d_pool.tile([P, 512], F32, tag="rec")
                                nc.vector.reciprocal(rec[:], psd[qt][:])
                                nc.vector.tensor_mul(
                                    ao16_sb[:, h, q0:q0 + 512], pso[qt][:], rec[:])

                # ===== output projection (transposed: w_o stationary) =====
                with ExitStack() as ctx:
                    oev_pool = ctx.enter_context(tc.tile_pool(name=f"oe{rep}", bufs=4))
                    ps_w = ctx.enter_context(
                        tc.tile_pool(name=f"apsw{rep}", bufs=8, space="PSUM"))
                    for hc in range(C.HID // P):
                        psw = [ps_w.tile([P, 512], F32, tag="psw", name=f"psw{i}")
                               for i in range(SC)]
                        for j in range(C.NH_G):
                            w_ap = wo_sb[:, j, hc * P:(hc + 1) * P]
                            for qc in range(SC):
                                nc.tensor.matmul(
                                    psw[qc][:], w_ap,
                                    ao16_sb[:, j, qc * 512:(qc + 1) * 512],
                                    start=(j == 0), stop=(j == C.NH_G - 1))
                        for qc in range(SC):
                            oev = oev_pool.tile([P, 512], BF16, tag="oev")
                            if qc % 2 == 0:
                                nc.vector.tensor_copy(oev[:], psw[qc][:])
                            else:
                                nc.scalar.copy(oev[:], psw[qc][:])
                            nc.sync.dma_start(
                                out=outpT[hc * P:(hc + 1) * P,
                                          qc * 512:(qc + 1) * 512],
                                in_=oev[:])

    nc.compile()
    return nc


def rope_tables(C: Cfg):
    """cos2/ssin2 [128, S] f32, two stacked 64-row blocks, pre-scaled by 1/WS."""
    inv = 1.0 / (10000.0 ** (np.arange(0, C.DR, 2, dtype=np.float64) / C.DR))
    freqs = np.arange(C.S, dtype=np.float64)[:, None] * inv[None, :]  # [S, 32]
    emb = np.concatenate([freqs, freqs], axis=1)  # [S, 64]
    cos = np.cos(emb).T
    sin = np.sin(emb).T
    ssin = sin.copy()
    ssin[: C.DR // 2] = -ssin[: C.DR // 2]
    cos2 = np.concatenate([cos, cos], axis=0) * IS
    ssin2 = np.concatenate([ssin, ssin], axis=0) * IS
    return (np.ascontiguousarray(cos2.astype(np.float32)),
            np.ascontiguousarray(ssin2.astype(np.float32)))


def swap32(w):
    """Swap the two 32-col halves of every 64-col group (rotate_half source)."""
    n = w.shape[1]
    return w.reshape(w.shape[0], n // 64, 2, 32)[:, :, ::-1, :].reshape(w.shape[0], n)


FP8NP = mybir.dt.np(FP8)
BF16NP = mybir.dt.np(BF16)


def host_inputs(C: Cfg, inputs: dict, core: int):
    NH = inputs["w_q_nope"].shape[1] // C.DN
    groups = NH // C.NH_G
    b = core // groups
    g = core % groups
    hs = slice(g * C.NH_G, (g + 1) * C.NH_G)

    f64 = lambda x: np.asarray(x, dtype=np.float64)
    fp8 = lambda x: np.ascontiguousarray(np.asarray(x, dtype=np.float32).astype(FP8NP))
    bf16 = lambda x: np.ascontiguousarray(
        np.asarray(x, dtype=np.float32).astype(BF16NP))

    hT = inputs["hidden_states"][b].T
    w_q_a = f64(inputs["w_q_a"])
    w_qbn = f64(inputs["w_q_nope"].reshape(C.QLR, NH, C.DN)[:, hs].reshape(C.QLR, -1))
    w_qbr = f64(inputs["w_q_rope"].reshape(C.QLR, NH, C.DR)[:, hs].reshape(C.QLR, -1))
    qn = w_q_a @ w_qbn          # [HID, 512]
    qr = w_q_a @ w_qbr          # [HID, 256]
    w_q8 = fp8(np.concatenate([qn, qr], axis=1) * WS)

    w_kbn = f64(inputs["w_k_nope"].reshape(C.KVLR, NH, C.DN)[:, hs].reshape(C.KVLR, -1))
    w_kbr = f64(inputs["w_k_rope"].reshape(C.KVLR, NH, C.DR)[:, hs].reshape(C.KVLR, -1))
    w_k8 = fp8(np.concatenate([w_kbn, w_kbr], axis=1) * WS)
    w_vb = bf16(inputs["w_v"].reshape(C.KVLR, NH, C.DV)[:, hs].reshape(C.KVLR, -1))

    w_ob = bf16(inputs["w_o"].reshape(NH, C.DV, C.HID)[hs].reshape(-1, C.HID))
    cos2, ssin2 = rope_tables(C)
    cm = np.asarray(inputs["causal_mask"])[0, 0]
    dmask = bf16(cm[-512:, -512:].T)
    return {
        "hT8": fp8(hT), "hT16": bf16(hT), "w_q8": w_q8,
        "w_kva": bf16(inputs["w_kv_a"]), "w_k8": w_k8, "w_vb": w_vb,
        "w_ob": w_ob, "cos2": cos2, "ssin2": ssin2, "dmask": dmask,
    }


_NC_CACHE = {}


def kernel(**inputs) -> np.ndarray:
    from concourse.bass_utils import run_bass_kernel_spmd

    C = CFG
    if "nc" not in _NC_CACHE:
        _NC_CACHE["nc"] = build_nc(C)
    nc = _NC_CACHE["nc"]

    in_maps = [host_inputs(C, inputs, c) for c in range(8)]
    res = run_bass_kernel_spmd(nc, in_maps, core_ids=list(range(8)))

    B = inputs["hidden_states"].shape[0]
    groups = 8 // B
    out = np.zeros((B, C.S, C.HID), dtype=np.float32)
    for c in range(8):
        out[c // groups] += np.asarray(res.results[c]["outpT"],
                                       dtype=np.float32).T
    return out
